# revision 23
# baseline (speedup 1.0000x reference)
"""Trainium2 Bass kernel for nn_LookupTableLayer (embedding_lookup).

Full-input contract: kernel(**inputs) takes the full unsharded numpy inputs,
shards positions across 8 NeuronCores (batch dim), runs one SPMD NEFF on
cores 0-7, and returns the full [16, 512, 32, 128] f32 output.

Algorithm:
  reference: t = 0.1*(table/max(table)) + fixed_table; gather rows at idx;
  concat(ex, ey) then reshape(...,128,2).sum(-1) == pair-sum of each gathered
  row. So out[..., 0:64] = pairsum(tx)[idx0], out[..., 64:128] = pairsum(ty)[idx1].
  We precompute the pair-summed 64-wide tables on-chip (1 MB each), store them
  to DRAM scratch, then gather 256 B rows with bulk InstDMAGatherAnt.

  Perf notes (measured on HW):
  - SWDGE descriptor generation on the Pool engine runs at ~8.5 ns/descriptor
    regardless of batching (dma_gather or indirect_dma_start), so the
    2*32768 = 65536 descriptors per core cost ~557 us and dominate. Every
    other engine is scheduled to hide under that stream; the remaining
    levers are the pre-gather ramp and the post-gather tail.
  - dma_gather is capped at 1024 descriptors per instruction (SWDGE ring),
    hence GL=1024 slices.
  - Index prep is per-chunk and pipelined (chunk k+1 prepped during chunk
    k's gathers) so chunk 0's prep is small and off the critical path.

  dma_gather semantics: idx element i of a chunk is read from
  idxs[i%16, i//16] (16-partition wrap, replicated across the 8 gpsimd core
  groups); its gathered row lands at out[i%128, i//128, :]. Positions are
  loaded so slot (p, c) holds token d*1024 + w*64 + c (p = d*16 + w), which
  makes the pos load 512 B-contiguous and the output store 32 KB-contiguous
  per partition.
"""

from contextlib import ExitStack

import numpy as np

import concourse.bacc as bacc
import concourse.bass as bass
import concourse.bass_isa as bass_isa
import concourse.mybir as mybir
import concourse.tile as tile
from concourse.bass_utils import run_bass_kernel_spmd

N_CORES = 8
B, M, R, D = 16, 512, 32, 128
TABLE_LEN = 4106
T = (B // N_CORES) * M * R  # 32768 tokens per core
PAIRS = D // 2  # 64
FLAT_N = TABLE_LEN * D // 128  # 4106 (flat table elems per partition)
PAIR_N = FLAT_N // 2  # 2053
CHUNK = 8192  # tokens per merge/store tile
NCHUNK = T // CHUNK  # 4
C = CHUNK // 128  # 64 gathered tokens per partition per chunk
GL = 1024  # tokens per dma_gather instruction (SWDGE ring = 1024 descs)
NG = CHUNK // GL  # 8 gather slices per chunk
GC = GL // 128  # 8 out columns per gather slice

F32 = mybir.dt.float32
I32 = mybir.dt.int32
I16 = mybir.dt.int16


def _flat(h, p):
    return h[:].rearrange("a b -> (a b)").rearrange("(p n) -> p n", p=p)


def build_nc():
    nc = bacc.Bacc("TRN2", target_bir_lowering=False, debug=False)
    pos = nc.dram_tensor("positions", [T, 2], I32, kind="ExternalInput")
    fixed = nc.dram_tensor("fixed_table", [TABLE_LEN, D], F32, kind="ExternalInput")
    tx = nc.dram_tensor("table_x", [TABLE_LEN, D], F32, kind="ExternalInput")
    ty = nc.dram_tensor("table_y", [TABLE_LEN, D], F32, kind="ExternalInput")
    out = nc.dram_tensor("out", [T, D], F32, kind="ExternalOutput")
    txp_d = nc.dram_tensor("txp", [TABLE_LEN, PAIRS], F32, kind="Internal")
    typ_d = nc.dram_tensor("typ", [TABLE_LEN, PAIRS], F32, kind="Internal")
    warm_d = nc.dram_tensor("warm", [128, PAIRS], F32, kind="Internal")

    with tile.TileContext(nc) as tc, ExitStack() as ctx:
        pwp = ctx.enter_context(tc.tile_pool(name="posw", bufs=2))
        ipp = ctx.enter_context(tc.tile_pool(name="idx", bufs=2))

        def idx_prep(k):
            """Load chunk k's positions in gather-wrap order and split the
            int32 (x, y) pairs into int16 idx tiles via bitcast."""
            posw = pwp.tile([128, 8, C, 2], I32, tag="posw")
            src = pos[k * CHUNK : (k + 1) * CHUNK, :].rearrange(
                "(d w c) j -> w d c j", d=8, w=16, c=C
            )
            for g in range(8):
                nc.sync.dma_start(posw[16 * g : 16 * (g + 1)], src)
            pxk = ipp.tile([128, C, 8], I16, tag="pxk")
            pyk = ipp.tile([128, C, 8], I16, tag="pyk")
            pw16 = posw[:].bitcast(I16)  # [128, 8, C, 4]
            nc.vector.tensor_copy(
                pxk[:].rearrange("p c (d one) -> p c d one", one=1),
                pw16[:, :, :, 0:1].rearrange("p d c one -> p c d one"),
            )
            nc.vector.tensor_copy(
                pyk[:].rearrange("p c (d one) -> p c d one", one=1),
                pw16[:, :, :, 2:3].rearrange("p d c one -> p c d one"),
            )
            return pxk, pyk

        with tc.tile_pool(name="prep", bufs=1) as prep:
            # ---- table preproc: txp = 0.1/max(x)*pairsum(x) + pairsum(fixed)
            # x chain first and tight so the txp store (the first gather's
            # dependency) lands as early as possible.
            xt = prep.tile([128, FLAT_N], F32)
            yt = prep.tile([128, FLAT_N], F32)
            ft = prep.tile([128, FLAT_N], F32)
            nc.sync.dma_start(xt[:], _flat(tx, 128))
            nc.sync.dma_start(ft[:], _flat(fixed, 128))
            nc.sync.dma_start(yt[:], _flat(ty, 128))
            idx0 = idx_prep(0)

            fp = prep.tile([128, PAIR_N], F32)
            fr = ft[:].rearrange("p (n two) -> p n two", two=2)

            def chain(src_t, dram, first):
                # reduce_max -> PAR runs on Pool while the vector engine does
                # the pairsums, so the scalar_tensor_tensor (and the store the
                # first gather waits on) lands as early as possible
                mx = prep.tile([128, 1], F32, tag="mx")
                nc.vector.reduce_max(mx[:], src_t[:], axis=mybir.AxisListType.X)
                gm = prep.tile([128, 1], F32, tag="gm")
                nc.gpsimd.partition_all_reduce(gm[:], mx[:], 128, bass_isa.ReduceOp.max)
                if first:
                    nc.vector.tensor_add(fp[:], fr[:, :, 0], fr[:, :, 1])
                pr = src_t[:].rearrange("p (n two) -> p n two", two=2)
                ps = prep.tile([128, PAIR_N], F32, tag="ps")
                nc.vector.tensor_add(ps[:], pr[:, :, 0], pr[:, :, 1])
                sc = prep.tile([128, 1], F32, tag="sc")
                nc.vector.reciprocal(sc[:], gm[:])
                nc.vector.tensor_scalar_mul(sc[:], sc[:], 0.1)
                nc.vector.scalar_tensor_tensor(
                    ps[:], ps[:], sc[:, 0:1], fp[:],
                    op0=mybir.AluOpType.mult, op1=mybir.AluOpType.add,
                )
                nc.sync.dma_start(_flat(dram, 128), ps[:])

            chain(xt, txp_d, True)
            # warmup gather on a dummy table during the ramp: the first real
            # dma_gather otherwise pays ~15 us of cold ucode/ring state
            warm_idx = prep.tile([128, GL // 16], I16, tag="warm_idx")
            nc.gpsimd.memset(warm_idx[:], 0)
            warm_g = prep.tile([128, GC, PAIRS], F32, tag="warm_g")
            nc.gpsimd.dma_gather(warm_g[:], warm_d[:], warm_idx[:], GL, GL, PAIRS)
            chain(yt, typ_d, False)

        # ---- main loop: sliced gathers/merges, pipelined idx prep,
        # progressively finer stores so the tail after the last gather is
        # only one slice of merge + a small store.
        gp = ctx.enter_context(tc.tile_pool(name="g", bufs=2))
        mp = ctx.enter_context(tc.tile_pool(name="m", bufs=2))
        idx_next = idx0
        for k in range(NCHUNK):
            pxk, pyk = idx_next
            gx = gp.tile([128, C, PAIRS], F32, tag="gx")
            gy = gp.tile([128, C, PAIRS], F32, tag="gy")
            mg = mp.tile([128, C, D], F32, tag="mg")
            idxv_x = pxk[:].rearrange("p c d -> p (c d)")
            idxv_y = pyk[:].rearrange("p c d -> p (c d)")
            oc = out[k * CHUNK : (k + 1) * CHUNK, :].rearrange(
                "(p c) f -> p c f", p=128
            )
            last = k == NCHUNK - 1
            # all x-slice gathers first: chunk 0's y gathers then start ~70 us
            # in, giving the typ preproc chain slack to finish off-path
            for j in range(NG):
                cs = slice(j * GC, (j + 1) * GC)
                nc.gpsimd.dma_gather(
                    gx[:, cs, :], txp_d[:],
                    idxv_x[:, j * GL // 16 : (j + 1) * GL // 16],
                    GL, GL, PAIRS,
                )
                nc.vector.tensor_copy(mg[:, cs, 0:PAIRS], gx[:, cs, :])
                # prefetch next chunk's indices mid-chunk: at j==0 the DMA
                # burst contends with the first gather's ring drain
                if j == 4 and k + 1 < NCHUNK:
                    idx_next = idx_prep(k + 1)
            # store boundaries (in y gather slices): coarse early, fine late
            bounds = [4, 6, 7, 8] if last else [4, 8]
            done = 0
            for j in range(NG):
                cs = slice(j * GC, (j + 1) * GC)
                nc.gpsimd.dma_gather(
                    gy[:, cs, :], typ_d[:],
                    idxv_y[:, j * GL // 16 : (j + 1) * GL // 16],
                    GL, GL, PAIRS,
                )
                nc.scalar.copy(mg[:, cs, PAIRS:D], gy[:, cs, :])
                if j + 1 in bounds:
                    c0, c1 = done * GC, (j + 1) * GC
                    nc.sync.dma_start(oc[:, c0:c1, :], mg[:, c0:c1, :])
                    done = j + 1

    nc.compile()
    return nc


_cache = {}


def kernel(positions, fixed_table, table_x, table_y):
    nc = _cache.get("nc")
    if nc is None:
        nc = _cache["nc"] = build_nc()
    pos_flat = np.ascontiguousarray(positions.reshape(-1, 2))
    shards = np.split(pos_flat, N_CORES, axis=0)
    fixed_table = np.ascontiguousarray(fixed_table, dtype=np.float32)
    table_x = np.ascontiguousarray(table_x, dtype=np.float32)
    table_y = np.ascontiguousarray(table_y, dtype=np.float32)
    in_maps = [
        {
            "positions": np.ascontiguousarray(s),
            "fixed_table": fixed_table,
            "table_x": table_x,
            "table_y": table_y,
        }
        for s in shards
    ]
    res = run_bass_kernel_spmd(nc, in_maps, core_ids=list(range(N_CORES)))
    outs = [r["out"] for r in res.results]
    return np.concatenate(outs, axis=0).reshape(B, M, R, D)
